# revision 1
# baseline (speedup 1.0000x reference)
"""CTC loss (keras ctc_batch_cost semantics) on 8 Trainium2 NeuronCores.

Algorithm: linear-space CTC forward DP, reformulated as a *wavefront* over
extended-label lanes.  For each label lane k the whole time axis is computed
with one hardware linear-recurrence instruction (tensor_tensor_scan on the
DVE), so the serial chain is over k (129 steps), not over t (512 steps).

  E[k]_t = pb_t * (E[k]_{t-1} + O[k-1]_{t-1})                 (blank state 2k)
  O[k]_t = pl[k]_t * (O[k]_{t-1} + E[k]_{t-1} + kap_k*O[k-1]_{t-1})  (label 2k+1)

Probabilities are pre-scaled by 1/r_t with r_t = sum_s p_s^2 / sum_s p_s
(self-weighted mean over extended states) so the linear-space values stay
inside fp32 range for all 512 steps; the loss adds back sum_t log r_t.

The per-(b,t) gather y_pred[b,t,y_true[b,k]] is done as a one-hot matmul on
the tensor engine; the [k,t]->[b,t] re-layout is a flat SBUF->SBUF DMA.
Batch is sharded 32 per core (pure data parallelism).
"""

import sys

for _p in ("/opt/trn_rl_repo",):
    if _p not in sys.path:
        sys.path.insert(0, _p)

from contextlib import ExitStack

import numpy as np

import concourse.bacc as bacc
import concourse.bass as bass
import concourse.tile as tile
from concourse import mybir
from concourse.bass_utils import run_bass_kernel_spmd

F32 = mybir.dt.float32
AF = mybir.ActivationFunctionType
OP = mybir.AluOpType

B, T, C, L = 256, 512, 256, 128
NCORES = 8
BS = B // NCORES
EPS = 1e-7
BLANK = C - 1

_nc_cache = {}


def build_nc(bs=BS, t=T, c=C, l=L):
    key = (bs, t, c, l)
    if key in _nc_cache:
        return _nc_cache[key]
    CT = c // 128
    GRP = min(8, bs)
    nc = bacc.Bacc("TRN2")
    ypT = nc.declare_dram_parameter("ypT", [bs, c, t], F32, isOutput=False)
    Gd = nc.declare_dram_parameter("G", [bs, c, l], F32, isOutput=False)
    cntd = nc.declare_dram_parameter("cnt", [bs, c, 1], F32, isOutput=False)
    kapd = nc.declare_dram_parameter("kap", [bs, l], F32, isOutput=False)
    lossd = nc.declare_dram_parameter("loss", [bs, 1], F32, isOutput=True)

    with ExitStack() as ctx:
        tc = ctx.enter_context(tile.TileContext(nc))
        pers = ctx.enter_context(tc.tile_pool(name="pers", bufs=1))
        ypool = ctx.enter_context(tc.tile_pool(name="y", bufs=2))
        gpool = ctx.enter_context(tc.tile_pool(name="g", bufs=2))
        y2pool = ctx.enter_context(tc.tile_pool(name="y2", bufs=3))
        bcpool = ctx.enter_context(tc.tile_pool(name="bc", bufs=3))
        pspool = ctx.enter_context(
            tc.tile_pool(name="ps", bufs=3, space=bass.MemorySpace.PSUM)
        )
        psspool = ctx.enter_context(
            tc.tile_pool(name="pss", bufs=2, space=bass.MemorySpace.PSUM)
        )
        drampool = ctx.enter_context(
            tc.tile_pool(name="dram", bufs=2, space=bass.MemorySpace.DRAM)
        )

        pl_big = pers.tile([128, bs * t], F32)  # scaled gathered label probs
        PB = pers.tile([bs, t], F32)
        INVR = pers.tile([bs, t], F32)
        PBS = pers.tile([bs, t], F32)
        KAP = pers.tile([bs, l], F32)
        LOGACC = pers.tile([bs, 1], F32)
        SCR = pers.tile([bs, t], F32)
        ZERO = pers.tile([bs, t], F32)
        FIN = pers.tile([bs, 1], F32)
        LLOG = pers.tile([bs, 1], F32)
        LOSS = pers.tile([bs, 1], F32)

        nc.sync.dma_start(KAP[:], kapd[:])
        nc.gpsimd.memset(ZERO[:], 0.0)

        # ---------------- phase A: gather + scaling, in groups of GRP ----
        for g0 in range(0, bs, GRP):
            ng = min(GRP, bs - g0)
            ytiles = {}
            # A1: load y, blank rows, squares, s1/s2 matmuls
            i1 = bcpool.tile([GRP, t], F32, tag="i1")
            iv = bcpool.tile([GRP, t], F32, tag="iv")
            for loc in range(ng):
                b = g0 + loc
                psg = psspool.tile([33, t], F32, tag="psg")
                cts = []
                for ci in range(CT):
                    y = ypool.tile([128, t], F32, tag=f"Y{loc}_{ci}")
                    nc.sync.dma_start(y[:], ypT[b, ci * 128 : (ci + 1) * 128, :])
                    ytiles[(loc, ci)] = y
                    cn = gpool.tile([128, 1], F32, tag=f"cn{ci}")
                    nc.sync.dma_start(cn[:], cntd[b, ci * 128 : (ci + 1) * 128, :])
                    cts.append(cn)
                nc.sync.dma_start(PB[b : b + 1, :], ypT[b, BLANK : BLANK + 1, :])
                for ci in range(CT):
                    y2 = y2pool.tile([128, t], F32, tag="Y2")
                    nc.scalar.activation(y2[:], ytiles[(loc, ci)][:], AF.Square)
                    nc.tensor.matmul(
                        psg[0:1, :],
                        cts[ci][:],
                        ytiles[(loc, ci)][:],
                        start=(ci == 0),
                        stop=(ci == CT - 1),
                    )
                    nc.tensor.matmul(
                        psg[32:33, :],
                        cts[ci][:],
                        y2[:],
                        start=(ci == 0),
                        stop=(ci == CT - 1),
                    )
                # evac s1/s2 rows via SBUF bounce (engines can't start at
                # partition b; DMA can)
                pse = y2pool.tile([33, t], F32, tag="pse")
                nc.scalar.copy(pse[0:1, :], psg[0:1, :])
                nc.scalar.copy(pse[32:33, :], psg[32:33, :])
                nc.sync.dma_start(i1[loc : loc + 1, :], pse[0:1, :])
                nc.sync.dma_start(iv[loc : loc + 1, :], pse[32:33, :])
            # invr = s1 / s2   (r = s2/s1 = selfweighted mean prob)
            nc.vector.reciprocal(iv[0:ng, :], iv[0:ng, :])
            nc.vector.tensor_mul(iv[0:ng, :], iv[0:ng, :], i1[0:ng, :])
            nc.sync.dma_start(INVR[g0 : g0 + ng, :], iv[0:ng, :])
            # A2: gather matmul + scaled evac
            for loc in range(ng):
                b = g0 + loc
                gts = []
                for ci in range(CT):
                    gt = gpool.tile([128, l], F32, tag=f"G{ci}")
                    nc.sync.dma_start(gt[:], Gd[b, ci * 128 : (ci + 1) * 128, :])
                    gts.append(gt)
                ps = pspool.tile([128, t], F32, tag="plps")
                for ci in range(CT):
                    nc.tensor.matmul(
                        ps[0:l, :],
                        gts[ci][:],
                        ytiles[(loc, ci)][:],
                        start=(ci == 0),
                        stop=(ci == CT - 1),
                    )
                ivd = drampool.tile([1, t], F32, tag="ivd")
                nc.sync.dma_start(ivd[:], iv[loc : loc + 1, :])
                bc = bcpool.tile([128, t], F32, tag="bc")
                nc.sync.dma_start(bc[:], ivd[:].to_broadcast((128, t)))
                # pl_big[0:l, b] = (ps + EPS) * invr_bcast
                nc.vector.scalar_tensor_tensor(
                    pl_big[0:l, b * t : (b + 1) * t],
                    ps[0:l, :],
                    float(EPS),
                    bc[0:l, :],
                    OP.add,
                    OP.mult,
                )
        # scaled blank probs + log-accumulator
        nc.vector.scalar_tensor_tensor(
            PBS[:], PB[:], float(EPS), INVR[:], OP.add, OP.mult
        )
        nc.scalar.activation(SCR[:], INVR[:], AF.Ln, accum_out=LOGACC[:])

        # ---------------- phase B+C: wavefront over label lanes ----------
        E0 = pers.tile([bs, 1 + t], F32)
        Ebuf = [pers.tile([bs, 1 + t], F32, name=f"Eb{i}") for i in range(2)]
        Obuf = [pers.tile([bs, 1 + t], F32, name=f"Ob{i}") for i in range(3)]
        Dbuf = [pers.tile([bs, t], F32, name=f"Db{i}") for i in range(2)]
        plbuf = [pers.tile([bs, t], F32, name=f"plb{i}") for i in range(4)]
        nc.gpsimd.memset(E0[:, 0:1], 1.0)
        for tb in Ebuf + Obuf:
            nc.gpsimd.memset(tb[:, 0:1], 0.0)

        def shuffle(k, dst):
            # row k of pl_big, b-blocks -> [bs, t]
            nc.sync.dma_start(dst[:], pl_big[k : k + 1, :])

        # k = 0
        nc.vector.tensor_tensor_scan(
            E0[:, 1 : 1 + t], ZERO[:], PBS[:], E0[:, 0:1], OP.add, OP.mult
        )
        shuffle(0, plbuf[0])
        nc.vector.tensor_tensor_scan(
            Obuf[0][:, 1 : 1 + t],
            E0[:, 0:t],
            plbuf[0][:],
            Obuf[0][:, 0:1],
            OP.add,
            OP.mult,
        )
        prevO = Obuf[0]
        for k in range(1, l):
            Ek = Ebuf[k % 2]
            Ok = Obuf[k % 3]
            dl = Dbuf[k % 2]
            plk = plbuf[k % 4]
            shuffle(k, plk)
            nc.vector.tensor_tensor_scan(
                Ek[:, 1 : 1 + t], prevO[:, 0:t], PBS[:], Ek[:, 0:1], OP.add, OP.mult
            )
            nc.vector.scalar_tensor_tensor(
                dl[:], prevO[:, 0:t], KAP[:, k : k + 1], Ek[:, 0:t], OP.mult, OP.add
            )
            nc.vector.tensor_tensor_scan(
                Ok[:, 1 : 1 + t], dl[:], plk[:], Ok[:, 0:1], OP.add, OP.mult
            )
            prevO = Ok
        EL = Ebuf[l % 2]
        nc.vector.tensor_tensor_scan(
            EL[:, 1 : 1 + t], prevO[:, 0:t], PBS[:], EL[:, 0:1], OP.add, OP.mult
        )
        nc.vector.tensor_add(FIN[:], EL[:, t : t + 1], prevO[:, t : t + 1])
        nc.scalar.activation(LLOG[:], FIN[:], AF.Ln)
        nc.vector.tensor_sub(LOSS[:], LOGACC[:], LLOG[:])
        nc.sync.dma_start(lossd[:], LOSS[:])

    nc.finalize()
    _nc_cache[key] = nc
    return nc


def host_prep(y_true, y_pred, bs=BS, t=T, c=C, l=L):
    """Per-core input maps: transposed probs, one-hot gather matrix, counts,
    skip mask."""
    ncores = y_true.shape[0] // bs
    maps = []
    for core in range(ncores):
        sl = slice(core * bs, (core + 1) * bs)
        yt = np.asarray(y_true[sl], dtype=np.int32)
        ypT = np.ascontiguousarray(
            np.asarray(y_pred[sl], dtype=np.float32).transpose(0, 2, 1)
        )
        G = (yt[:, None, :] == np.arange(c, dtype=np.int32)[None, :, None]).astype(
            np.float32
        )
        cnt = G.sum(axis=2, keepdims=True)
        cnt[:, c - 1, 0] = l + 1.0  # blank multiplicity in extended states
        kap = np.zeros((bs, l), dtype=np.float32)
        kap[:, 1:] = (yt[:, 1:] != yt[:, :-1]).astype(np.float32)
        maps.append({"ypT": ypT, "G": G, "cnt": cnt, "kap": kap})
    return maps


def kernel(y_true, y_pred):
    nc = build_nc()
    maps = host_prep(y_true, y_pred)
    res = run_bass_kernel_spmd(nc, maps, list(range(NCORES)))
    loss = np.concatenate([res.results[i]["loss"] for i in range(NCORES)], axis=0)
    return loss.astype(np.float32)



# revision 11
# speedup vs baseline: 1.1771x; 1.1771x over previous
"""CTC loss (keras ctc_batch_cost semantics) on 8 Trainium2 NeuronCores.

Linear-space CTC forward DP as a *packed wavefront* over extended-label lanes
and time blocks.  The time axis T=512 is split into NG=4 blocks of W=128; the
partition dim packs (block j, batch b) = 4*32 = 128 partitions.  Diagonal step
d computes lane k = d - j for every block j simultaneously, so each hardware
scan is only W elements long and the serial chain is L+1+NG-1 = 132 steps of
[128, 128] ops instead of 129 steps of [32, 512] ops.

  E[k]_t = pb_t * (E[k]_{t-1} + O[k-1]_{t-1})                 (blank state 2k)
  O[k]_t = pl[k]_t * (O[k]_{t-1} + E[k]_{t-1} + kap_k*O[k-1]_{t-1})  (label 2k+1)

Carries between partition groups (block j-1 -> j) are 3 scalars per partition
(E/O/dl block-ends), moved by a 128x128 shift matmul on the otherwise idle
tensor engine; the scans read their initial value straight from PSUM.

Probabilities are pre-scaled by 1/r_t with r_t = sum_s p_s^2 / sum_s p_s over
extended states so linear-space values stay in fp32 range; the loss adds back
sum_t log r_t.  r is computed from the *gathered* label rows via a ones-vector
matmul (no separate count-weighted stats over all 256 classes).

The per-(b,t) gather y_pred[b,t,y_true[b,k]] is a one-hot matmul in bf16.
Batch is sharded 32 per core (pure data parallelism).
"""

import sys

for _p in ("/opt/trn_rl_repo",):
    if _p not in sys.path:
        sys.path.insert(0, _p)

from contextlib import ExitStack

import numpy as np
import ml_dtypes

import concourse.bacc as bacc
import concourse.bass as bass
import concourse.tile as tile
from concourse import mybir
from concourse.bass_utils import run_bass_kernel_spmd

F32 = mybir.dt.float32
BF16 = mybir.dt.bfloat16
AF = mybir.ActivationFunctionType
OP = mybir.AluOpType

B, T, C, L = 256, 512, 256, 128
NCORES = 8
BS = B // NCORES
EPS = 1e-7
BLANK = C - 1

_nc_cache = {}


def build_nc(bs=BS, t=T, c=C, l=L):
    key = (bs, t, c, l)
    if key in _nc_cache:
        return _nc_cache[key]
    CT = c // 128
    NG = 4
    W = t // NG
    P = NG * bs
    NSTEP = (l + 1) + NG - 1
    nc = bacc.Bacc("TRN2")
    ypT = nc.declare_dram_parameter("ypT", [bs, c, t], BF16, isOutput=False)
    Gd = nc.declare_dram_parameter("G", [bs, c, l], BF16, isOutput=False)
    kapdd = nc.declare_dram_parameter("kapd", [P, NSTEP], F32, isOutput=False)
    shwd = nc.declare_dram_parameter("shw", [P, P], F32, isOutput=False)
    lossd = nc.declare_dram_parameter("loss", [bs, 1], F32, isOutput=True)

    with ExitStack() as ctx:
        tc = ctx.enter_context(tile.TileContext(nc))
        pers = ctx.enter_context(tc.tile_pool(name="pers", bufs=1))
        ypool = ctx.enter_context(tc.tile_pool(name="y", bufs=3))
        gpool = ctx.enter_context(tc.tile_pool(name="g", bufs=3))
        y2pool = ctx.enter_context(tc.tile_pool(name="y2", bufs=3))
        evpool = ctx.enter_context(tc.tile_pool(name="ev", bufs=3))
        pspool = ctx.enter_context(
            tc.tile_pool(name="ps", bufs=2, space=bass.MemorySpace.PSUM)
        )
        psspool = ctx.enter_context(
            tc.tile_pool(name="pss", bufs=2, space=bass.MemorySpace.PSUM)
        )
        shpool = ctx.enter_context(
            tc.tile_pool(name="sh", bufs=2, space=bass.MemorySpace.PSUM)
        )

        # persistent state
        pl_big = pers.tile([128, bs, t], BF16)  # raw gathered label probs
        PB = pers.tile([bs, t], BF16)  # raw blank probs
        PB2 = pers.tile([bs, t], F32)
        I1 = pers.tile([bs, t], F32)
        I2 = pers.tile([bs, t], F32)
        INVR = pers.tile([bs, t], F32)
        PBSI = pers.tile([bs, t], F32)
        SCR = pers.tile([bs, t], F32)
        LOGACC = pers.tile([bs, 1], F32)
        PBSP = pers.tile([P, W], F32)  # packed (pb+EPS)*invr per group-block
        INVRPK = pers.tile([P, W], F32)  # packed invr per group-block
        KAPD = pers.tile([P, NSTEP], F32)
        SHW = pers.tile([P, P], F32)
        ONES = pers.tile([128, 1], BF16)
        EIN0 = pers.tile([P, 1], F32)
        OIN0 = pers.tile([P, 1], F32)
        FEO = pers.tile([bs, 2], F32)
        FIN = pers.tile([bs, 1], F32)
        LLOG = pers.tile([bs, 1], F32)
        LOSS = pers.tile([bs, 1], F32)
        Ebuf = [pers.tile([P, W], F32, name=f"Eb{i}") for i in range(2)]
        Obuf = [pers.tile([P, 1 + W], F32, name=f"Ob{i}") for i in range(2)]
        Dbuf = [pers.tile([P, 1 + W], F32, name=f"Db{i}") for i in range(2)]
        plbuf = [pers.tile([P, W], BF16, name=f"plb{i}") for i in range(4)]
        plsbuf = [pers.tile([P, W], F32, name=f"plsb{i}") for i in range(4)]

        nc.sync.dma_start(KAPD[:], kapdd[:])
        nc.sync.dma_start(SHW[:], shwd[:])
        nc.gpsimd.memset(ONES[:], 1.0)
        nc.gpsimd.memset(EIN0[:], 0.0)
        nc.gpsimd.memset(EIN0[0:bs, :], 1.0)
        nc.gpsimd.memset(OIN0[:], 0.0)
        for tb in Ebuf + Obuf + Dbuf + plbuf + plsbuf:
            nc.gpsimd.memset(tb[:], 0.0)
        # dl_{-1} for lane 0 is 1 (E[0]_{-1}=1): col0 of Dbuf[0], group 0
        nc.gpsimd.memset(Dbuf[0][0:bs, 0:1], 1.0)

        # ---------------- phase A: gather + r stats, bf16 ----------------
        for b in range(bs):
            ytiles = []
            for ci in range(CT):
                y = ypool.tile([128, t], BF16, tag=f"Y{ci}")
                nc.sync.dma_start(y[:], ypT[b, ci * 128 : (ci + 1) * 128, :])
                ytiles.append(y)
            nc.sync.dma_start(PB[b : b + 1, :], ypT[b, BLANK : BLANK + 1, :])
            gts = []
            for ci in range(CT):
                gt = gpool.tile([128, l], BF16, tag=f"G{ci}")
                nc.sync.dma_start(gt[:], Gd[b, ci * 128 : (ci + 1) * 128, :])
                gts.append(gt)
            ps = pspool.tile([128, t], F32, tag="plps")
            for ci in range(CT):
                nc.tensor.matmul(
                    ps[0:l, :],
                    gts[ci][:],
                    ytiles[ci][:],
                    start=(ci == 0),
                    stop=(ci == CT - 1),
                )
            # evac gathered probs (+EPS folded in) to bf16
            nc.scalar.activation(
                pl_big[0:l, b : b + 1, :], ps[0:l, :], AF.Copy, bias=float(EPS)
            )
            # lane-sum stats from the gathered rows
            pl2 = y2pool.tile([128, t], BF16, tag="pl2")
            nc.scalar.activation(pl2[0:l, :], pl_big[0:l, b : b + 1, :], AF.Square)
            psg = psspool.tile([33, t], F32, tag="psg")
            nc.tensor.matmul(psg[0:1, :], ONES[0:l, :], pl_big[0:l, b : b + 1, :])
            nc.tensor.matmul(psg[32:33, :], ONES[0:l, :], pl2[0:l, :])
            pse = evpool.tile([33, t], F32, tag="pse")
            nc.scalar.copy(pse[0:1, :], psg[0:1, :])
            nc.scalar.copy(pse[32:33, :], psg[32:33, :])
            nc.sync.dma_start(I1[b : b + 1, :], pse[0:1, :])
            nc.sync.dma_start(I2[b : b + 1, :], pse[32:33, :])

        # r_t = s2/s1 over extended states; s includes (l+1) blank copies
        nc.scalar.activation(PB2[:], PB[:], AF.Square)
        nc.vector.scalar_tensor_tensor(I1[:], PB[:], float(l + 1), I1[:], OP.mult, OP.add)
        nc.vector.scalar_tensor_tensor(I2[:], PB2[:], float(l + 1), I2[:], OP.mult, OP.add)
        nc.vector.reciprocal(I2[:], I2[:])
        nc.vector.tensor_mul(INVR[:], I2[:], I1[:])  # invr = s1/s2
        nc.scalar.activation(SCR[:], INVR[:], AF.Ln, accum_out=LOGACC[:])
        # (pb+EPS)*invr, then pack per (group j = block j)
        nc.vector.scalar_tensor_tensor(PBSI[:], PB[:], float(EPS), INVR[:], OP.add, OP.mult)
        for j in range(NG):
            nc.sync.dma_start(PBSP[j * bs : (j + 1) * bs, :], PBSI[:, j * W : (j + 1) * W])
            nc.sync.dma_start(INVRPK[j * bs : (j + 1) * bs, :], INVR[:, j * W : (j + 1) * W])

        # ---------------- phase B: packed wavefront ----------------------
        sh_tiles = {}
        for d in range(NSTEP):
            # prefetch + scale pl for this step's (lane, block) cells
            plp = plbuf[d % 4]
            any_lane = False
            for j in range(NG):
                k = d - j
                if 0 <= k < l:
                    nc.sync.dma_start(
                        plp[j * bs : (j + 1) * bs, :],
                        pl_big[k : k + 1, 0:bs, j * W : (j + 1) * W],
                    )
                    any_lane = True
            pls = plsbuf[d % 4]
            if any_lane:
                nc.gpsimd.tensor_tensor(pls[:], plp[:], INVRPK[:], OP.mult)

            Ek = Ebuf[d % 2]
            Ok = Obuf[d % 2]
            Dk = Dbuf[d % 2]
            Oprev = Obuf[(d - 1) % 2]
            if d == 0:
                einit = EIN0[:, 0:1]
                oinit = OIN0[:, 0:1]
            else:
                sh_prev = sh_tiles[(d - 1) % 2]
                einit = sh_prev[:, 0:1]
                oinit = sh_prev[:, 1:2]
            nc.vector.tensor_tensor_scan(
                Ek[:, 0:W], Oprev[:, 0:W], PBSP[:], einit, OP.add, OP.mult
            )
            if d < NSTEP - 1:
                nc.vector.scalar_tensor_tensor(
                    Dk[:, 1 : 1 + W], Oprev[:, 1 : 1 + W], KAPD[:, d : d + 1],
                    Ek[:, 0:W], OP.mult, OP.add,
                )
                nc.vector.tensor_tensor_scan(
                    Ok[:, 1 : 1 + W], Dk[:, 0:W], pls[:], oinit, OP.add, OP.mult
                )
            if d < NSTEP - 1:
                sh = shpool.tile([P, 4], F32, tag="sh")
                sh_tiles[d % 2] = sh
                nc.tensor.matmul(sh[:, 0:1], SHW[:], Ek[:, W - 1 : W])
                nc.tensor.matmul(sh[:, 1:2], SHW[:], Ok[:, W : W + 1])
                nc.tensor.matmul(sh[:, 2:3], SHW[:], Dk[:, W : W + 1])
                # seed next step's shifted-in first elements
                nc.scalar.copy(Obuf[(d + 1) % 2][:, 0:1], sh[:, 1:2])
                nc.scalar.copy(Dbuf[(d + 1) % 2][:, 0:1], sh[:, 2:3])

        # results live in group NG-1 (partitions [P-bs:P])
        nc.sync.dma_start(
            FEO[:, 0:1], Obuf[(NSTEP - 2) % 2][P - bs : P, W : W + 1]
        )
        nc.sync.dma_start(
            FEO[:, 1:2], Ebuf[(NSTEP - 1) % 2][P - bs : P, W - 1 : W]
        )
        nc.vector.tensor_add(FIN[:], FEO[:, 0:1], FEO[:, 1:2])
        nc.scalar.activation(LLOG[:], FIN[:], AF.Ln)
        nc.vector.tensor_sub(LOSS[:], LOGACC[:], LLOG[:])
        nc.sync.dma_start(lossd[:], LOSS[:])

    nc.finalize()
    _nc_cache[key] = nc
    return nc


def host_prep(y_true, y_pred, bs=BS, t=T, c=C, l=L):
    """Per-core input maps: transposed bf16 probs, one-hot gather matrix,
    packed skip mask, shift matrix."""
    NG = 4
    P = NG * bs
    NSTEP = (l + 1) + NG - 1
    ncores = y_true.shape[0] // bs
    shw = np.zeros((P, P), dtype=np.float32)
    for p in range(P - bs):
        shw[p, p + bs] = 1.0
    maps = []
    for core in range(ncores):
        sl = slice(core * bs, (core + 1) * bs)
        yt = np.asarray(y_true[sl], dtype=np.int32)
        ypT = np.ascontiguousarray(
            np.asarray(y_pred[sl], dtype=np.float32).transpose(0, 2, 1)
        ).astype(ml_dtypes.bfloat16)
        G = (yt[:, None, :] == np.arange(c, dtype=np.int32)[None, :, None]).astype(
            ml_dtypes.bfloat16
        )
        kap = np.zeros((bs, l), dtype=np.float32)
        kap[:, 1:] = (yt[:, 1:] != yt[:, :-1]).astype(np.float32)
        kapd = np.zeros((P, NSTEP), dtype=np.float32)
        for j in range(NG):
            for d in range(NSTEP):
                k = d - j
                if 0 <= k < l:
                    kapd[j * bs : (j + 1) * bs, d] = kap[:, k]
        maps.append({"ypT": ypT, "G": G, "kapd": kapd, "shw": shw})
    return maps


def kernel(y_true, y_pred):
    nc = build_nc()
    maps = host_prep(y_true, y_pred)
    res = run_bass_kernel_spmd(nc, maps, list(range(NCORES)))
    loss = np.concatenate([res.results[i]["loss"] for i in range(NCORES)], axis=0)
    return loss.astype(np.float32)


# revision 13
# speedup vs baseline: 1.5271x; 1.2974x over previous
"""CTC loss (keras ctc_batch_cost semantics) on 8 Trainium2 NeuronCores.

Linear-space CTC forward DP as a *packed wavefront* over extended-label lanes
and time blocks.  T=512 is split into NG=4 blocks of W=128; the partition dim
packs (block j, batch b) = 4*32 = 128 partitions.  Diagonal step d computes
lane k = d - j for every block j simultaneously: each scan is W elements and
the serial chain is L+1+NG-1 = 132 steps of [128, W] ops.

  E[k]_t = pb_t * (E[k]_{t-1} + O[k-1]_{t-1})                 (blank state 2k)
  O[k]_t = pl[k]_t * (O[k]_{t-1} + E[k]_{t-1} + kap_k*O[k-1]_{t-1})  (label 2k+1)

Per step the E/O/dl state lives in ONE chain tile CH with segments laid out at
uniform stride W+2, so the block-end carries (group j-1 -> j) move with two
tiny shift matmuls on the idle tensor engine (E-end right after scanE; O/D
ends as a 2-column strided AP after scanO); scans read their initial value
straight from the PSUM the shift lands in.

All per-step pl operands are pre-packed into one big SBUF tile (PLALL) by DMAs
that overlap phase A, so phase B issues no DMAs at all.  Probabilities are
pre-scaled by 1/r_t (r_t = sum_s p_s^2 / sum_s p_s over extended states,
computed from the gathered rows via ones-matmuls); the loss adds back
sum_t log r_t.  Everything data-sized is bf16; batch is sharded 32 per core.
"""

import sys

for _p in ("/opt/trn_rl_repo",):
    if _p not in sys.path:
        sys.path.insert(0, _p)

from contextlib import ExitStack

import numpy as np
import ml_dtypes

import concourse.bacc as bacc
import concourse.bass as bass
import concourse.tile as tile
from concourse import mybir
from concourse.bass_utils import run_bass_kernel_spmd

F32 = mybir.dt.float32
BF16 = mybir.dt.bfloat16
AF = mybir.ActivationFunctionType
OP = mybir.AluOpType

B, T, C, L = 256, 512, 256, 128
NCORES = 8
BS = B // NCORES
EPS = 1e-7
BLANK = C - 1

_nc_cache = {}


def build_nc(bs=BS, t=T, c=C, l=L):
    key = (bs, t, c, l)
    if key in _nc_cache:
        return _nc_cache[key]
    CT = c // 128
    NG = 4
    W = t // NG
    P = NG * bs
    NSTEP = (l + 1) + NG - 1
    # chain tile column layout: E [0:W), O [W+1:2W+2), D [2W+3:3W+4)
    # last cols: E=W-1, O=2W+1, D=3W+3  -> uniform stride W+2
    OC = W + 1      # O segment start (carry col); outputs at [OC+1, OC+1+W)
    DC = 2 * W + 3  # D segment start (carry col); outputs at [DC+1, DC+1+W)
    CHW = 3 * W + 4
    nc = bacc.Bacc("TRN2")
    ypT = nc.declare_dram_parameter("ypT", [bs, c, t], BF16, isOutput=False)
    Gd = nc.declare_dram_parameter("G", [c, bs * l], BF16, isOutput=False)
    pbd = nc.declare_dram_parameter("pb", [bs, t], BF16, isOutput=False)
    kapdd = nc.declare_dram_parameter("kapd", [P, NSTEP], F32, isOutput=False)
    shwd = nc.declare_dram_parameter("shw", [P, P], BF16, isOutput=False)
    lossd = nc.declare_dram_parameter("loss", [bs, 1], F32, isOutput=True)

    with ExitStack() as ctx:
        tc = ctx.enter_context(tile.TileContext(nc))
        pers = ctx.enter_context(tc.tile_pool(name="pers", bufs=1))
        ypool = ctx.enter_context(tc.tile_pool(name="y", bufs=3))
        y2pool = ctx.enter_context(tc.tile_pool(name="y2", bufs=3))
        pspool = ctx.enter_context(
            tc.tile_pool(name="ps", bufs=2, space=bass.MemorySpace.PSUM)
        )
        psspool = ctx.enter_context(
            tc.tile_pool(name="pss", bufs=2, space=bass.MemorySpace.PSUM)
        )
        shpool = ctx.enter_context(
            tc.tile_pool(name="sh", bufs=2, space=bass.MemorySpace.PSUM)
        )

        # persistent state
        pl_big = pers.tile([128, bs, t], BF16)  # gathered label probs (+EPS)
        PLALL = pers.tile([P, NSTEP * W], BF16)  # skew-packed per-step pl
        GT = [pers.tile([128, bs * l], BF16, name=f"GT{ci}") for ci in range(CT)]
        PB = pers.tile([bs, t], BF16)
        PB2 = pers.tile([bs, t], F32)
        PSE = pers.tile([33, bs * t], BF16)  # staging rows for s1/s2
        I1 = pers.tile([bs, t], BF16)
        I2 = pers.tile([bs, t], BF16)
        S1 = pers.tile([bs, t], F32)
        S2 = pers.tile([bs, t], F32)
        INVR = pers.tile([bs, t], F32)
        PBSI = pers.tile([bs, t], F32)
        SCR = pers.tile([bs, t], F32)
        LOGACC = pers.tile([bs, 1], F32)
        PBSP = pers.tile([P, W], F32)
        INVRPK = pers.tile([P, W], F32)
        KAPD = pers.tile([P, NSTEP], F32)
        SHW = pers.tile([P, P], BF16)
        ONES = pers.tile([128, 1], BF16)
        EIN0 = pers.tile([P, 1], F32)
        OIN0 = pers.tile([P, 1], F32)
        FEO = pers.tile([bs, 2], BF16)
        FIN = pers.tile([bs, 1], F32)
        LLOG = pers.tile([bs, 1], F32)
        LOSS = pers.tile([bs, 1], F32)
        CH = [pers.tile([P, CHW], BF16, name=f"CH{i}") for i in range(2)]
        plsbuf = [pers.tile([P, W], BF16, name=f"plsb{i}") for i in range(6)]

        nc.sync.dma_start(KAPD[:], kapdd[:])
        nc.sync.dma_start(SHW[:], shwd[:])
        nc.sync.dma_start(PB[:], pbd[:])
        for ci in range(CT):
            nc.sync.dma_start(GT[ci][:], Gd[ci * 128 : (ci + 1) * 128, :])
        nc.gpsimd.memset(ONES[:], 1.0)
        nc.gpsimd.memset(EIN0[:], 0.0)
        nc.gpsimd.memset(EIN0[0:bs, :], 1.0)
        nc.gpsimd.memset(OIN0[:], 0.0)
        nc.gpsimd.memset(PLALL[:], 0.0)
        for tb in CH + plsbuf:
            nc.gpsimd.memset(tb[:], 0.0)
        # dl_{-1} for lane 0 is 1 (E[0]_{-1}=1): D carry col, group 0
        nc.gpsimd.memset(CH[0][0:bs, DC : DC + 1], 1.0)

        # ---------------- phase A: gather + r stats, bf16 ----------------
        pack_engines = [nc.gpsimd, nc.scalar, nc.gpsimd, nc.scalar]
        for b in range(bs):
            ytiles = []
            for ci in range(CT):
                y = ypool.tile([128, t], BF16, tag=f"Y{ci}")
                nc.sync.dma_start(y[:], ypT[b, ci * 128 : (ci + 1) * 128, :])
                ytiles.append(y)
            ps = pspool.tile([128, t], F32, tag="plps")
            for ci in range(CT):
                nc.tensor.matmul(
                    ps[0:l, :],
                    GT[ci][:, b * l : (b + 1) * l],
                    ytiles[ci][:],
                    start=(ci == 0),
                    stop=(ci == CT - 1),
                )
            # evac gathered probs (+EPS folded in) to bf16
            nc.scalar.activation(
                pl_big[0:l, b : b + 1, :], ps[0:l, :], AF.Copy, bias=float(EPS)
            )
            # skew-pack this batch row into PLALL (overlaps with compute)
            for j in range(NG):
                eng = pack_engines[j]
                row = j * bs + b
                eng.dma_start(
                    PLALL[row : row + 1, j * W : (j + l) * W],
                    pl_big[0:l, b : b + 1, j * W : (j + 1) * W],
                )
            # lane-sum stats from the gathered rows
            pl2 = y2pool.tile([128, t], BF16, tag="pl2")
            nc.vector.tensor_tensor(
                pl2[0:l, :], pl_big[0:l, b : b + 1, :],
                pl_big[0:l, b : b + 1, :], OP.mult,
            )
            psg = psspool.tile([33, t], F32, tag="psg")
            nc.tensor.matmul(psg[0:1, :], ONES[0:l, :], pl_big[0:l, b : b + 1, :])
            nc.tensor.matmul(psg[32:33, :], ONES[0:l, :], pl2[0:l, :])
            nc.scalar.copy(PSE[0:1, b * t : (b + 1) * t], psg[0:1, :])
            nc.scalar.copy(PSE[32:33, b * t : (b + 1) * t], psg[32:33, :])
        nc.sync.dma_start(I1[:], PSE[0:1, :])
        nc.sync.dma_start(I2[:], PSE[32:33, :])

        # r_t = s2/s1 over extended states; s includes (l+1) blank copies
        nc.scalar.activation(PB2[:], PB[:], AF.Square)
        nc.vector.scalar_tensor_tensor(S1[:], PB[:], float(l + 1), I1[:], OP.mult, OP.add)
        nc.vector.scalar_tensor_tensor(S2[:], PB2[:], float(l + 1), I2[:], OP.mult, OP.add)
        nc.vector.reciprocal(S2[:], S2[:])
        nc.vector.tensor_mul(INVR[:], S2[:], S1[:])  # invr = s1/s2
        nc.scalar.activation(SCR[:], INVR[:], AF.Ln, accum_out=LOGACC[:])
        # (pb+EPS)*invr, then pack per (group j = block j)
        nc.vector.scalar_tensor_tensor(PBSI[:], PB[:], float(EPS), INVR[:], OP.add, OP.mult)
        for j in range(NG):
            nc.sync.dma_start(PBSP[j * bs : (j + 1) * bs, :], PBSI[:, j * W : (j + 1) * W])
            nc.sync.dma_start(INVRPK[j * bs : (j + 1) * bs, :], INVR[:, j * W : (j + 1) * W])

        # ---------------- phase B: packed wavefront ----------------------
        sh_tiles = {}
        for d in range(NSTEP):
            pls = plsbuf[d % 6]
            if d < NSTEP - 1:
                nc.gpsimd.tensor_tensor(
                    pls[:], PLALL[:, d * W : (d + 1) * W], INVRPK[:], OP.mult
                )
            ch = CH[d % 2]
            chp = CH[(d - 1) % 2]
            if d == 0:
                einit = EIN0[:, 0:1]
                oinit = OIN0[:, 0:1]
            else:
                sh_prev = sh_tiles[(d - 1) % 2]
                einit = sh_prev[:, 0:1]
                oinit = sh_prev[:, 1:2]
            # E[k] over this block
            nc.vector.tensor_tensor_scan(
                ch[:, 0:W], chp[:, OC : OC + W], PBSP[:], einit, OP.add, OP.mult
            )
            if d < NSTEP - 1:
                sh = shpool.tile([P, 4], F32, tag="sh")
                sh_tiles[d % 2] = sh
                # E block-end shifts early (hides under stt+scanO)
                nc.tensor.matmul(sh[:, 0:1], SHW[:], ch[:, W - 1 : W])
                # dl = kap*O[k-1] + E[k]
                nc.vector.scalar_tensor_tensor(
                    ch[:, DC + 1 : DC + 1 + W], chp[:, OC + 1 : OC + 1 + W],
                    KAPD[:, d : d + 1], ch[:, 0:W], OP.mult, OP.add,
                )
                # O[k] over this block
                nc.vector.tensor_tensor_scan(
                    ch[:, OC + 1 : OC + 1 + W], ch[:, DC : DC + W], pls[:],
                    oinit, OP.add, OP.mult,
                )
                # O and D block-ends as one strided 2-col matmul
                nc.tensor.matmul(
                    sh[:, 1:3], SHW[:], ch[:, 2 * W + 1 : CHW : W + 2]
                )
                # seed next step's shifted-in first elements
                nc.scalar.copy(CH[(d + 1) % 2][:, OC : OC + 1], sh[:, 1:2])
                nc.scalar.copy(CH[(d + 1) % 2][:, DC : DC + 1], sh[:, 2:3])

        # results live in group NG-1 (partitions [P-bs:P])
        nc.sync.dma_start(
            FEO[:, 0:1], CH[(NSTEP - 2) % 2][P - bs : P, 2 * W + 1 : 2 * W + 2]
        )
        nc.sync.dma_start(
            FEO[:, 1:2], CH[(NSTEP - 1) % 2][P - bs : P, W - 1 : W]
        )
        nc.vector.tensor_add(FIN[:], FEO[:, 0:1], FEO[:, 1:2])
        nc.scalar.activation(LLOG[:], FIN[:], AF.Ln)
        nc.vector.tensor_sub(LOSS[:], LOGACC[:], LLOG[:])
        nc.sync.dma_start(lossd[:], LOSS[:])

    nc.finalize()
    _nc_cache[key] = nc
    return nc


def host_prep(y_true, y_pred, bs=BS, t=T, c=C, l=L):
    """Per-core input maps: transposed bf16 probs, one-hot gather matrix laid
    out [c, bs*l], blank rows, packed skip mask, shift matrix."""
    NG = 4
    P = NG * bs
    NSTEP = (l + 1) + NG - 1
    ncores = y_true.shape[0] // bs
    shw = np.zeros((P, P), dtype=ml_dtypes.bfloat16)
    for p in range(P - bs):
        shw[p, p + bs] = 1.0
    maps = []
    for core in range(ncores):
        sl = slice(core * bs, (core + 1) * bs)
        yt = np.asarray(y_true[sl], dtype=np.int32)
        ypT = np.ascontiguousarray(
            np.asarray(y_pred[sl], dtype=np.float32).transpose(0, 2, 1)
        ).astype(ml_dtypes.bfloat16)
        pb = np.ascontiguousarray(ypT[:, c - 1, :])
        # G[c, b*l] one-hot: G[cc, b*l + k] = (yt[b, k] == cc)
        G = (
            (yt[None, :, :] == np.arange(c, dtype=np.int32)[:, None, None])
            .astype(ml_dtypes.bfloat16)
            .reshape(c, bs * l)
        )
        kap = np.zeros((bs, l), dtype=np.float32)
        kap[:, 1:] = (yt[:, 1:] != yt[:, :-1]).astype(np.float32)
        kapd = np.zeros((P, NSTEP), dtype=np.float32)
        for j in range(NG):
            for d in range(NSTEP):
                k = d - j
                if 0 <= k < l:
                    kapd[j * bs : (j + 1) * bs, d] = kap[:, k]
        maps.append({"ypT": ypT, "G": G, "pb": pb, "kapd": kapd, "shw": shw})
    return maps


def kernel(y_true, y_pred):
    nc = build_nc()
    maps = host_prep(y_true, y_pred)
    res = run_bass_kernel_spmd(nc, maps, list(range(NCORES)))
    loss = np.concatenate([res.results[i]["loss"] for i in range(NCORES)], axis=0)
    return loss.astype(np.float32)
